# Initial kernel scaffold
#
"""Trainium2 Bass kernel for the AHGCSP GCN layer problem.

Computes, per batch element b (8 total, one per NeuronCore):
    F   = Dynamic_L[b] * W[b,:,:,0] + Geo * W[b,:,:,1] + KL * W[b,:,:,2]
    P   = softmax(F, axis=-1)
    G1  = P @ inputs[b]
    out = tanh(G1 @ Wd + bd)

Formulation on device (everything transposed host-side, free for HW time):
  - Stream m-tiles of F^T = DL^T*W0^T + Geo^T*W1^T + KL^T*W2^T  [128 m, 2048 r]
  - P^T = exp(F^T)  (no max subtraction; F is bounded ~|8|)
  - G1T_aug[f',r] = sum_m Xaug[m,f'] * P^T[m,r] accumulated in PSUM, where
    Xaug = [inputs[b] | ones] so row 64 of G1T_aug is the softmax denominator.
  - normalize with reciprocal + partition_broadcast, second matmul with Wd as
    stationary producing out^T [64 u, 2048 r], tanh(+bd bias) fused on ScalarE.
  - host transposes out^T back.
"""

import numpy as np

import bass_rust
import concourse.bass as bass
import concourse.mybir as mybir
from concourse.tile import TileContext
from concourse.vector_clock import ScopedClock
from concourse.bass_utils import run_bass_kernel_spmd

B, N, F, UNITS = 8, 2048, 64, 64
P = 128            # partitions
MT = N // P        # m-tiles per core
FA = F + 1         # augmented feature dim (ones column)
NQ = 4             # moving-dim quarters (N / 512)
QW = N // NQ       # 512

FP32 = mybir.dt.float32


class SplitDrainTileContext(TileContext):
    """TileContext whose exit drain is split into single-wait drains.

    The stock exit emits one SP drain carrying one sync-wait per outstanding
    logical proc; walrus in this toolchain caps non-EventSemaphore
    instructions at 1 sync wait, so chain single-wait drains instead.
    """

    def _drain_and_barrier(self, tick_clock, wait_clock):
        drain_inst = self.nc.sync.drain()
        wait_clock.add_sem_waits(
            drain_inst.ins, ScopedClock({None: tick_clock.global_clock})
        )
        waits = list(drain_inst.ins.sync_info.on_wait)
        updates = list(drain_inst.ins.sync_info.on_update)
        if len(waits) > 1:
            drain_inst.ins.sync_info = bass_rust.SyncInfo(
                on_wait=waits[:1], on_update=updates
            )
            for w in waits[1:]:
                d2 = self.nc.sync.drain()
                d2.ins.sync_info = bass_rust.SyncInfo(on_wait=[w], on_update=[])

        self.nc.all_engine_barrier()
        assert self.sems is not None
        popped = self.nc._tile_sem_poison_stack.pop()
        assert popped is self._sem_poison
        self.nc.clear_and_free_semaphores(list(self.sems.allocated().values()))
        self.nc.all_engine_barrier()


def build_nc(passes: int = 1, in_bufs: int = 2, work_bufs: int = 2):
    """Build the per-core Bass graph. `passes` repeats the whole computation
    (for slope-based wall-clock timing); output is identical each pass."""
    nc = bass.Bass(num_devices=B)

    dlt = nc.declare_dram_parameter("dlt", [N, N], FP32, isOutput=False)
    w0t = nc.declare_dram_parameter("w0t", [N, N], FP32, isOutput=False)
    w1t = nc.declare_dram_parameter("w1t", [N, N], FP32, isOutput=False)
    w2t = nc.declare_dram_parameter("w2t", [N, N], FP32, isOutput=False)
    geot = nc.declare_dram_parameter("geot", [N, N], FP32, isOutput=False)
    klt = nc.declare_dram_parameter("klt", [N, N], FP32, isOutput=False)
    xperm = nc.declare_dram_parameter("xperm", [P, MT * FA], FP32, isOutput=False)
    wd = nc.declare_dram_parameter("wd", [F, UNITS], FP32, isOutput=False)
    bdt = nc.declare_dram_parameter("bdt", [UNITS, 1], FP32, isOutput=False)
    outT = nc.declare_dram_parameter("outT", [UNITS, N], FP32, isOutput=True)

    with SplitDrainTileContext(nc) as tc:
        with (
            tc.tile_pool(name="consts", bufs=1) as cpool,
            tc.tile_pool(name="ins", bufs=in_bufs) as ipool,
            tc.tile_pool(name="work", bufs=work_bufs) as wpool,
            tc.tile_pool(name="epi", bufs=1) as epool,
            tc.tile_pool(name="psum", bufs=1, space="PSUM") as ppool,
        ):
            x_sbuf = cpool.tile([P, MT * FA], FP32, tag="x")
            nc.sync.dma_start(out=x_sbuf[:, :], in_=xperm[:, :])
            wd_sbuf = cpool.tile([F, UNITS], FP32, tag="wd")
            nc.sync.dma_start(out=wd_sbuf[:, :], in_=wd[:, :])
            bd_sbuf = cpool.tile([UNITS, 1], FP32, tag="bd")
            nc.sync.dma_start(out=bd_sbuf[:, :], in_=bdt[:, :])

            for _ in range(passes):
                psum_g1 = ppool.tile([FA, N], FP32, tag="g1")
                for mi in range(MT):
                    rs = slice(P * mi, P * (mi + 1))
                    dl = ipool.tile([P, N], FP32, tag="dl")
                    nc.sync.dma_start(out=dl[:, :], in_=dlt[rs, :])
                    w0 = ipool.tile([P, N], FP32, tag="w0")
                    nc.sync.dma_start(out=w0[:, :], in_=w0t[rs, :])
                    w1 = ipool.tile([P, N], FP32, tag="w1")
                    nc.sync.dma_start(out=w1[:, :], in_=w1t[rs, :])
                    w2 = ipool.tile([P, N], FP32, tag="w2")
                    nc.sync.dma_start(out=w2[:, :], in_=w2t[rs, :])
                    geo = ipool.tile([P, N], FP32, tag="geo")
                    nc.sync.dma_start(out=geo[:, :], in_=geot[rs, :])
                    kl = ipool.tile([P, N], FP32, tag="kl")
                    nc.sync.dma_start(out=kl[:, :], in_=klt[rs, :])

                    ft = wpool.tile([P, N], FP32, tag="ft")
                    tmp = wpool.tile([P, N], FP32, tag="tmp")
                    nc.vector.tensor_mul(ft[:, :], dl[:, :], w0[:, :])
                    nc.vector.tensor_mul(tmp[:, :], geo[:, :], w1[:, :])
                    nc.vector.tensor_add(ft[:, :], ft[:, :], tmp[:, :])
                    nc.vector.tensor_mul(tmp[:, :], kl[:, :], w2[:, :])
                    nc.vector.tensor_add(ft[:, :], ft[:, :], tmp[:, :])

                    pt = wpool.tile([P, N], FP32, tag="pt")
                    nc.scalar.activation(
                        pt[:, :], ft[:, :], mybir.ActivationFunctionType.Exp
                    )

                    xa = x_sbuf[:, FA * mi : FA * (mi + 1)]
                    for q in range(NQ):
                        nc.tensor.matmul(
                            psum_g1[:, QW * q : QW * (q + 1)],
                            xa,
                            pt[:, QW * q : QW * (q + 1)],
                            start=(mi == 0),
                            stop=(mi == MT - 1),
                        )

                # epilogue
                g1t = epool.tile([FA, N], FP32, tag="g1t")
                nc.scalar.copy(g1t[:, :], psum_g1[:, :])
                recip = epool.tile([1, N], FP32, tag="recip")
                nc.vector.reciprocal_approx_fast(
                    out=recip[:, :], in_=g1t[F : F + 1, :]
                )
                recip_b = epool.tile([F, N], FP32, tag="recip_b")
                nc.gpsimd.partition_broadcast(recip_b[:, :], recip[:, :])
                g1n = epool.tile([F, N], FP32, tag="g1n")
                nc.vector.tensor_mul(g1n[:, :], g1t[:F, :], recip_b[:, :])

                psum_h = ppool.tile([UNITS, N], FP32, tag="h")
                for q in range(NQ):
                    nc.tensor.matmul(
                        psum_h[:, QW * q : QW * (q + 1)],
                        wd_sbuf[:, :],
                        g1n[:, QW * q : QW * (q + 1)],
                        start=True,
                        stop=True,
                    )
                outt = epool.tile([UNITS, N], FP32, tag="outt")
                nc.scalar.activation(
                    outt[:, :],
                    psum_h[:, :],
                    mybir.ActivationFunctionType.Tanh,
                    bias=bd_sbuf[:, :],
                )
                nc.sync.dma_start(out=outT[:, :], in_=outt[:, :])

    return nc


def prepare_in_maps(inputs, Dynamic_L, W, Geo, KL, Wd, bd):
    """Host-side sharding + layout transforms (not counted in HW time)."""
    inputs = np.ascontiguousarray(inputs, dtype=np.float32)
    Dynamic_L = np.asarray(Dynamic_L, dtype=np.float32)
    W = np.asarray(W, dtype=np.float32)
    geot = np.ascontiguousarray(np.asarray(Geo, dtype=np.float32).T)
    klt = np.ascontiguousarray(np.asarray(KL, dtype=np.float32).T)
    wd = np.ascontiguousarray(np.asarray(Wd, dtype=np.float32))
    bdt = np.ascontiguousarray(np.asarray(bd, dtype=np.float32).reshape(UNITS, 1))

    in_maps = []
    for b in range(B):
        xaug = np.concatenate(
            [inputs[b], np.ones((N, 1), dtype=np.float32)], axis=1
        )  # [N, FA]
        xperm = np.ascontiguousarray(
            xaug.reshape(MT, P, FA).transpose(1, 0, 2).reshape(P, MT * FA)
        )
        in_maps.append(
            {
                "dlt": np.ascontiguousarray(Dynamic_L[b].T),
                "w0t": np.ascontiguousarray(W[b, :, :, 0].T),
                "w1t": np.ascontiguousarray(W[b, :, :, 1].T),
                "w2t": np.ascontiguousarray(W[b, :, :, 2].T),
                "geot": geot,
                "klt": klt,
                "xperm": xperm,
                "wd": wd,
                "bdt": bdt,
            }
        )
    return in_maps


_NC_CACHE = {}


def _get_nc(passes=1):
    if passes not in _NC_CACHE:
        _NC_CACHE[passes] = build_nc(passes=passes)
    return _NC_CACHE[passes]


def kernel(**inputs) -> np.ndarray:
    in_maps = prepare_in_maps(**inputs)
    nc = _get_nc(passes=1)
    res = run_bass_kernel_spmd(nc, in_maps, core_ids=list(range(B)))
    out = np.stack([res.results[b]["outT"].T for b in range(B)], axis=0)
    return np.ascontiguousarray(out, dtype=np.float32)


if __name__ == "__main__":
    rng = np.random.default_rng(0)
    ins = {
        "inputs": rng.standard_normal((B, N, F), dtype=np.float32),
        "Dynamic_L": rng.standard_normal((B, N, N), dtype=np.float32),
        "W": rng.random((B, N, N, 3), dtype=np.float32),
        "Geo": rng.standard_normal((N, N), dtype=np.float32),
        "KL": rng.standard_normal((N, N), dtype=np.float32),
        "Wd": rng.standard_normal((F, UNITS), dtype=np.float32) / 8.0,
        "bd": np.zeros(UNITS, dtype=np.float32),
    }
    out = kernel(**ins)
    print("out", out.shape, out.dtype)


# revision 14
# speedup vs baseline: 1.7600x; 1.7600x over previous
"""Trainium2 Bass kernel for the AHGCSP GCN layer problem.

Computes, per batch element b (8 total, one per NeuronCore):
    F   = Dynamic_L[b] * W[b,:,:,0] + Geo * W[b,:,:,1] + KL * W[b,:,:,2]
    P   = softmax(F, axis=-1)
    G1  = P @ inputs[b]
    out = tanh(G1 @ Wd + bd)

Formulation on device (everything transposed host-side, free for HW time):
  - Stream m-tiles of F^T = DL^T*W0^T + Geo^T*W1^T + KL^T*W2^T  [128 m, 2048 r]
  - P^T = exp(F^T)  (no max subtraction; F is bounded ~|8|)
  - G1T_aug[f',r] = sum_m Xaug[m,f'] * P^T[m,r] accumulated in PSUM, where
    Xaug = [inputs[b] | ones] so row 64 of G1T_aug is the softmax denominator.
  - 1/denom = exp(-ln(denom)) on ScalarE; broadcast across partitions via a
    K=1 matmul against a ones column; normalize on VectorE; second matmul with
    Wd stationary producing out^T [64 u, 2048 r]; tanh(+bd bias) on ScalarE.
  - host transposes out^T back.
"""

import numpy as np

import bass_rust
import concourse.bass as bass
import concourse.mybir as mybir
from concourse.tile import TileContext
from concourse.bass_utils import run_bass_kernel_spmd

B, N, F, UNITS = 8, 2048, 64, 64
P = 128            # partitions
MT = N // P        # m-tiles per core
FA = F + 1         # augmented feature dim (ones column)
NQ = 4             # moving-dim quarters (N / 512)
QW = N // NQ       # 512

FP32 = mybir.dt.float32
BF16 = mybir.dt.bfloat16
USE_BF16 = True          # cast the six big inputs (and X) to bf16 host-side
DT_IN = BF16 if USE_BF16 else FP32


def _cap_sync_waits(nc, max_waits=1):
    """The walrus build in this toolchain rejects instructions carrying more
    than a couple of sync waits ("Too many sync wait commands"). Hoist excess
    waits onto freshly inserted same-engine drain instructions immediately
    preceding the offender — identical blocking semantics, legal encoding."""
    eng_map = {
        mybir.EngineType.PE: nc.tensor,
        mybir.EngineType.DVE: nc.vector,
        mybir.EngineType.Activation: nc.scalar,
        mybir.EngineType.Pool: nc.gpsimd,
        mybir.EngineType.SP: nc.sync,
    }

    def _steal_fresh_drain(eng):
        binst = eng.drain()
        dmi = binst.ins
        for bb2 in nc.main_func.blocks:
            l2 = bb2.instructions
            if l2 and l2[-1].name == dmi.name:
                l2.pop()
                return dmi
        raise RuntimeError("could not find freshly appended drain")

    for bb in nc.main_func.blocks:
        il = bb.instructions
        i = 0
        while i < len(il):
            inst = il[i]
            si = inst.sync_info
            if si is not None and len(si.on_wait) > max_waits:
                waits = list(si.on_wait)
                extra, keep = waits[:-max_waits], waits[-max_waits:]
                eng = eng_map[inst.engine]
                for j in range(0, len(extra), max_waits):
                    dmi = _steal_fresh_drain(eng)
                    dmi.sync_info = bass_rust.SyncInfo(
                        on_wait=extra[j : j + max_waits], on_update=[]
                    )
                    il.insert(i, dmi)
                    i += 1
                inst.sync_info = bass_rust.SyncInfo(
                    on_wait=keep, on_update=list(si.on_update)
                )
            i += 1


def build_nc(passes: int = 1, in_bufs: int = 4, work_bufs: int = 2):
    """Build the per-core Bass graph. `passes` repeats the whole computation
    (for slope-based wall-clock timing); output is identical each pass."""
    nc = bass.Bass(num_devices=B)

    dlt = nc.declare_dram_parameter("dlt", [N, N], DT_IN, isOutput=False)
    w0t = nc.declare_dram_parameter("w0t", [N, N], DT_IN, isOutput=False)
    w1t = nc.declare_dram_parameter("w1t", [N, N], DT_IN, isOutput=False)
    w2t = nc.declare_dram_parameter("w2t", [N, N], DT_IN, isOutput=False)
    geot = nc.declare_dram_parameter("geot", [N, N], DT_IN, isOutput=False)
    klt = nc.declare_dram_parameter("klt", [N, N], DT_IN, isOutput=False)
    xperm = nc.declare_dram_parameter("xperm", [P, MT * FA], FP32, isOutput=False)
    wd = nc.declare_dram_parameter("wd", [F, UNITS], FP32, isOutput=False)
    bdt = nc.declare_dram_parameter("bdt", [UNITS, 1], FP32, isOutput=False)
    outT = nc.declare_dram_parameter("outT", [UNITS, N], FP32, isOutput=True)

    with TileContext(nc) as tc:
        with (
            tc.tile_pool(name="consts", bufs=1) as cpool,
            tc.tile_pool(name="ins", bufs=in_bufs) as ipool,
            tc.tile_pool(name="work", bufs=work_bufs) as wpool,
            tc.tile_pool(name="epi", bufs=1) as epool,
            tc.tile_pool(name="psum", bufs=1, space="PSUM") as ppool,
        ):
            x_sbuf = cpool.tile([P, MT * FA], FP32, tag="x")
            nc.sync.dma_start(out=x_sbuf[:, :], in_=xperm[:, :])
            wd_sbuf = cpool.tile([F, UNITS], FP32, tag="wd")
            nc.sync.dma_start(out=wd_sbuf[:, :], in_=wd[:, :])
            bd_sbuf = cpool.tile([UNITS, 1], FP32, tag="bd")
            nc.sync.dma_start(out=bd_sbuf[:, :], in_=bdt[:, :])
            ones_sb = cpool.tile([1, UNITS], FP32, tag="ones")
            nc.vector.memset(ones_sb[:, :], 1.0)

            for _ in range(passes):
                psum_g1 = ppool.tile([FA, N], FP32, tag="g1")
                for mi in range(MT):
                    rs = slice(P * mi, P * (mi + 1))
                    # pack DL|Geo|KL and W0|W1|W2 side by side so one wide
                    # tensor_tensor computes all three products
                    a3 = ipool.tile([P, 3 * N], DT_IN, tag="a3")
                    nc.sync.dma_start(out=a3[:, 0:N], in_=dlt[rs, :])
                    nc.sync.dma_start(out=a3[:, N : 2 * N], in_=geot[rs, :])
                    nc.sync.dma_start(out=a3[:, 2 * N : 3 * N], in_=klt[rs, :])
                    w3 = ipool.tile([P, 3 * N], DT_IN, tag="w3")
                    nc.sync.dma_start(out=w3[:, 0:N], in_=w0t[rs, :])
                    nc.sync.dma_start(out=w3[:, N : 2 * N], in_=w1t[rs, :])
                    nc.sync.dma_start(out=w3[:, 2 * N : 3 * N], in_=w2t[rs, :])

                    prod = wpool.tile([P, 3 * N], DT_IN, tag="prod")
                    nc.vector.tensor_mul(prod[:, :], a3[:, :], w3[:, :])
                    ft = wpool.tile([P, N], DT_IN, tag="ft")
                    nc.vector.tensor_add(
                        ft[:, :], prod[:, 0:N], prod[:, N : 2 * N]
                    )
                    nc.vector.tensor_add(ft[:, :], ft[:, :], prod[:, 2 * N : 3 * N])

                    pt = wpool.tile([P, N], FP32, tag="pt")
                    nc.scalar.activation(
                        pt[:, :], ft[:, :], mybir.ActivationFunctionType.Exp
                    )

                    xa = x_sbuf[:, FA * mi : FA * (mi + 1)]
                    for q in range(NQ):
                        nc.tensor.matmul(
                            psum_g1[:, QW * q : QW * (q + 1)],
                            xa,
                            pt[:, QW * q : QW * (q + 1)],
                            start=(mi == 0),
                            stop=(mi == MT - 1),
                        )

                # epilogue
                # DVE copies G1 rows while ScalarE derives 1/denom from the
                # PSUM denominator row directly: recip = exp(-ln(denom)).
                g1t = epool.tile([F, N], FP32, tag="g1t")
                nc.vector.tensor_copy(g1t[:, :], psum_g1[:F, :])
                lnd = epool.tile([1, N], FP32, tag="lnd")
                nc.scalar.activation(
                    lnd[:, :], psum_g1[F : F + 1, :], mybir.ActivationFunctionType.Ln
                )
                recip = epool.tile([1, N], FP32, tag="recip")
                nc.scalar.activation(
                    recip[:, :],
                    lnd[:, :],
                    mybir.ActivationFunctionType.Exp,
                    scale=-1.0,
                )
                # broadcast recip across partitions via K=1 matmul with ones
                psum_bc = ppool.tile([F, N], FP32, tag="bc")
                for q in range(NQ):
                    nc.tensor.matmul(
                        psum_bc[:, QW * q : QW * (q + 1)],
                        ones_sb[:, :F],
                        recip[:, QW * q : QW * (q + 1)],
                        start=True,
                        stop=True,
                    )
                g1n = epool.tile([F, N], FP32, tag="g1n")
                nc.vector.tensor_mul(g1n[:, :], g1t[:, :], psum_bc[:, :])

                psum_h = ppool.tile([UNITS, N], FP32, tag="g1")
                for q in range(NQ):
                    nc.tensor.matmul(
                        psum_h[:, QW * q : QW * (q + 1)],
                        wd_sbuf[:, :],
                        g1n[:, QW * q : QW * (q + 1)],
                        start=True,
                        stop=True,
                    )
                outt = epool.tile([UNITS, N], FP32, tag="outt")
                nc.scalar.activation(
                    outt[:, :],
                    psum_h[:, :],
                    mybir.ActivationFunctionType.Tanh,
                    bias=bd_sbuf[:, :],
                )
                nc.sync.dma_start(out=outT[:, :], in_=outt[:, :])

    _cap_sync_waits(nc)
    return nc


def prepare_in_maps(inputs, Dynamic_L, W, Geo, KL, Wd, bd):
    """Host-side sharding + layout transforms (not counted in HW time)."""
    import ml_dtypes

    dt_in = ml_dtypes.bfloat16 if USE_BF16 else np.float32
    inputs = np.ascontiguousarray(inputs, dtype=np.float32)
    Dynamic_L = np.asarray(Dynamic_L, dtype=np.float32)
    W = np.asarray(W, dtype=np.float32)
    geot = np.ascontiguousarray(np.asarray(Geo, dtype=np.float32).T).astype(dt_in)
    klt = np.ascontiguousarray(np.asarray(KL, dtype=np.float32).T).astype(dt_in)
    wd = np.ascontiguousarray(np.asarray(Wd, dtype=np.float32))
    bdt = np.ascontiguousarray(np.asarray(bd, dtype=np.float32).reshape(UNITS, 1))

    in_maps = []
    for b in range(B):
        xaug = np.concatenate(
            [inputs[b], np.ones((N, 1), dtype=np.float32)], axis=1
        )  # [N, FA]
        xperm = np.ascontiguousarray(
            xaug.reshape(MT, P, FA).transpose(1, 0, 2).reshape(P, MT * FA)
        )
        in_maps.append(
            {
                "dlt": np.ascontiguousarray(Dynamic_L[b].T).astype(dt_in),
                "w0t": np.ascontiguousarray(W[b, :, :, 0].T).astype(dt_in),
                "w1t": np.ascontiguousarray(W[b, :, :, 1].T).astype(dt_in),
                "w2t": np.ascontiguousarray(W[b, :, :, 2].T).astype(dt_in),
                "geot": geot,
                "klt": klt,
                "xperm": xperm,
                "wd": wd,
                "bdt": bdt,
            }
        )
    return in_maps


_NC_CACHE = {}


def _get_nc(passes=1):
    if passes not in _NC_CACHE:
        _NC_CACHE[passes] = build_nc(passes=passes)
    return _NC_CACHE[passes]


def kernel(**inputs) -> np.ndarray:
    in_maps = prepare_in_maps(**inputs)
    nc = _get_nc(passes=1)
    res = run_bass_kernel_spmd(nc, in_maps, core_ids=list(range(B)))
    out = np.stack([res.results[b]["outT"].T for b in range(B)], axis=0)
    return np.ascontiguousarray(out, dtype=np.float32)


if __name__ == "__main__":
    rng = np.random.default_rng(0)
    ins = {
        "inputs": rng.standard_normal((B, N, F), dtype=np.float32),
        "Dynamic_L": rng.standard_normal((B, N, N), dtype=np.float32),
        "W": rng.random((B, N, N, 3), dtype=np.float32),
        "Geo": rng.standard_normal((N, N), dtype=np.float32),
        "KL": rng.standard_normal((N, N), dtype=np.float32),
        "Wd": rng.standard_normal((F, UNITS), dtype=np.float32) / 8.0,
        "bd": np.zeros(UNITS, dtype=np.float32),
    }
    out = kernel(**ins)
    print("out", out.shape, out.dtype)



# revision 15
# speedup vs baseline: 1.7703x; 1.0058x over previous
"""Trainium2 Bass kernel for the AHGCSP GCN layer problem.

Computes, per batch element b (8 total, one per NeuronCore):
    F   = Dynamic_L[b] * W[b,:,:,0] + Geo * W[b,:,:,1] + KL * W[b,:,:,2]
    P   = softmax(F, axis=-1)
    G1  = P @ inputs[b]
    out = tanh(G1 @ Wd + bd)

Formulation on device (everything transposed host-side, free for HW time):
  - Stream m-tiles of F^T = DL^T*W0^T + Geo^T*W1^T + KL^T*W2^T  [128 m, 2048 r]
  - P^T = exp(F^T)  (no max subtraction; F is bounded ~|8|)
  - G1T_aug[f',r] = sum_m Xaug[m,f'] * P^T[m,r] accumulated in PSUM, where
    Xaug = [inputs[b] | ones] so row 64 of G1T_aug is the softmax denominator.
  - 1/denom = exp(-ln(denom)) on ScalarE; broadcast across partitions via a
    K=1 matmul against a ones column; normalize on VectorE; second matmul with
    Wd stationary producing out^T [64 u, 2048 r]; tanh(+bd bias) on ScalarE.
  - host transposes out^T back.
"""

import numpy as np

import bass_rust
import concourse.bass as bass
import concourse.mybir as mybir
from concourse.tile import TileContext
from concourse.bass_utils import run_bass_kernel_spmd

B, N, F, UNITS = 8, 2048, 64, 64
P = 128            # partitions
MT = N // P        # m-tiles per core
FA = F + 1         # augmented feature dim (ones column)
NQ = 4             # moving-dim quarters (N / 512)
QW = N // NQ       # 512

FP32 = mybir.dt.float32
BF16 = mybir.dt.bfloat16
USE_BF16 = True          # cast the six big inputs (and X) to bf16 host-side
DT_IN = BF16 if USE_BF16 else FP32


def _cap_sync_waits(nc, max_waits=1):
    """The walrus build in this toolchain rejects instructions carrying more
    than a couple of sync waits ("Too many sync wait commands"). Hoist excess
    waits onto freshly inserted same-engine drain instructions immediately
    preceding the offender — identical blocking semantics, legal encoding."""
    eng_map = {
        mybir.EngineType.PE: nc.tensor,
        mybir.EngineType.DVE: nc.vector,
        mybir.EngineType.Activation: nc.scalar,
        mybir.EngineType.Pool: nc.gpsimd,
        mybir.EngineType.SP: nc.sync,
    }

    def _steal_fresh_drain(eng):
        binst = eng.drain()
        dmi = binst.ins
        for bb2 in nc.main_func.blocks:
            l2 = bb2.instructions
            if l2 and l2[-1].name == dmi.name:
                l2.pop()
                return dmi
        raise RuntimeError("could not find freshly appended drain")

    for bb in nc.main_func.blocks:
        il = bb.instructions
        i = 0
        while i < len(il):
            inst = il[i]
            si = inst.sync_info
            if si is not None and len(si.on_wait) > max_waits:
                waits = list(si.on_wait)
                extra, keep = waits[:-max_waits], waits[-max_waits:]
                eng = eng_map[inst.engine]
                for j in range(0, len(extra), max_waits):
                    dmi = _steal_fresh_drain(eng)
                    dmi.sync_info = bass_rust.SyncInfo(
                        on_wait=extra[j : j + max_waits], on_update=[]
                    )
                    il.insert(i, dmi)
                    i += 1
                inst.sync_info = bass_rust.SyncInfo(
                    on_wait=keep, on_update=list(si.on_update)
                )
            i += 1


def build_nc(passes: int = 1, in_bufs: int = 5, work_bufs: int = 2):
    """Build the per-core Bass graph. `passes` repeats the whole computation
    (for slope-based wall-clock timing); output is identical each pass."""
    nc = bass.Bass(num_devices=B)

    dlt = nc.declare_dram_parameter("dlt", [N, N], DT_IN, isOutput=False)
    w0t = nc.declare_dram_parameter("w0t", [N, N], DT_IN, isOutput=False)
    w1t = nc.declare_dram_parameter("w1t", [N, N], DT_IN, isOutput=False)
    w2t = nc.declare_dram_parameter("w2t", [N, N], DT_IN, isOutput=False)
    geot = nc.declare_dram_parameter("geot", [N, N], DT_IN, isOutput=False)
    klt = nc.declare_dram_parameter("klt", [N, N], DT_IN, isOutput=False)
    xperm = nc.declare_dram_parameter("xperm", [P, MT * FA], FP32, isOutput=False)
    wd = nc.declare_dram_parameter("wd", [F, UNITS], FP32, isOutput=False)
    bdt = nc.declare_dram_parameter("bdt", [UNITS, 1], FP32, isOutput=False)
    outT = nc.declare_dram_parameter("outT", [UNITS, N], FP32, isOutput=True)

    with TileContext(nc) as tc:
        with (
            tc.tile_pool(name="consts", bufs=1) as cpool,
            tc.tile_pool(name="ins", bufs=in_bufs) as ipool,
            tc.tile_pool(name="work", bufs=work_bufs) as wpool,
            tc.tile_pool(name="epi", bufs=1) as epool,
            tc.tile_pool(name="psum", bufs=1, space="PSUM") as ppool,
        ):
            x_sbuf = cpool.tile([P, MT * FA], FP32, tag="x")
            nc.sync.dma_start(out=x_sbuf[:, :], in_=xperm[:, :])
            wd_sbuf = cpool.tile([F, UNITS], FP32, tag="wd")
            nc.sync.dma_start(out=wd_sbuf[:, :], in_=wd[:, :])
            bd_sbuf = cpool.tile([UNITS, 1], FP32, tag="bd")
            nc.sync.dma_start(out=bd_sbuf[:, :], in_=bdt[:, :])
            ones_sb = cpool.tile([1, UNITS], FP32, tag="ones")
            nc.vector.memset(ones_sb[:, :], 1.0)

            for _ in range(passes):
                psum_g1 = ppool.tile([FA, N], FP32, tag="g1")
                for mi in range(MT):
                    rs = slice(P * mi, P * (mi + 1))
                    # pack DL|Geo|KL and W0|W1|W2 side by side so one wide
                    # tensor_tensor computes all three products
                    a3 = ipool.tile([P, 3 * N], DT_IN, tag="a3")
                    nc.sync.dma_start(out=a3[:, 0:N], in_=dlt[rs, :])
                    nc.sync.dma_start(out=a3[:, N : 2 * N], in_=geot[rs, :])
                    nc.sync.dma_start(out=a3[:, 2 * N : 3 * N], in_=klt[rs, :])
                    w3 = ipool.tile([P, 3 * N], DT_IN, tag="w3")
                    nc.sync.dma_start(out=w3[:, 0:N], in_=w0t[rs, :])
                    nc.sync.dma_start(out=w3[:, N : 2 * N], in_=w1t[rs, :])
                    nc.sync.dma_start(out=w3[:, 2 * N : 3 * N], in_=w2t[rs, :])

                    prod = wpool.tile([P, 3 * N], DT_IN, tag="prod")
                    nc.vector.tensor_mul(prod[:, :], a3[:, :], w3[:, :])
                    # accumulate the three products in place (frees an SBUF tag)
                    nc.vector.tensor_add(
                        prod[:, 0:N], prod[:, 0:N], prod[:, N : 2 * N]
                    )
                    nc.vector.tensor_add(
                        prod[:, 0:N], prod[:, 0:N], prod[:, 2 * N : 3 * N]
                    )

                    pt = wpool.tile([P, N], FP32, tag="pt")
                    nc.scalar.activation(
                        pt[:, :], prod[:, 0:N], mybir.ActivationFunctionType.Exp
                    )

                    xa = x_sbuf[:, FA * mi : FA * (mi + 1)]
                    for q in range(NQ):
                        nc.tensor.matmul(
                            psum_g1[:, QW * q : QW * (q + 1)],
                            xa,
                            pt[:, QW * q : QW * (q + 1)],
                            start=(mi == 0),
                            stop=(mi == MT - 1),
                        )

                # epilogue
                # DVE copies G1 rows while ScalarE derives 1/denom from the
                # PSUM denominator row directly: recip = exp(-ln(denom)).
                g1t = epool.tile([F, N], FP32, tag="g1t")
                nc.vector.tensor_copy(g1t[:, :], psum_g1[:F, :])
                lnd = epool.tile([1, N], FP32, tag="lnd")
                nc.scalar.activation(
                    lnd[:, :], psum_g1[F : F + 1, :], mybir.ActivationFunctionType.Ln
                )
                recip = epool.tile([1, N], FP32, tag="recip")
                nc.scalar.activation(
                    recip[:, :],
                    lnd[:, :],
                    mybir.ActivationFunctionType.Exp,
                    scale=-1.0,
                )
                # broadcast recip across partitions via K=1 matmul with ones
                psum_bc = ppool.tile([F, N], FP32, tag="bc")
                for q in range(NQ):
                    nc.tensor.matmul(
                        psum_bc[:, QW * q : QW * (q + 1)],
                        ones_sb[:, :F],
                        recip[:, QW * q : QW * (q + 1)],
                        start=True,
                        stop=True,
                    )
                g1n = epool.tile([F, N], FP32, tag="g1n")
                nc.vector.tensor_mul(g1n[:, :], g1t[:, :], psum_bc[:, :])

                psum_h = ppool.tile([UNITS, N], FP32, tag="g1")
                for q in range(NQ):
                    nc.tensor.matmul(
                        psum_h[:, QW * q : QW * (q + 1)],
                        wd_sbuf[:, :],
                        g1n[:, QW * q : QW * (q + 1)],
                        start=True,
                        stop=True,
                    )
                outt = epool.tile([UNITS, N], FP32, tag="outt")
                nc.scalar.activation(
                    outt[:, :],
                    psum_h[:, :],
                    mybir.ActivationFunctionType.Tanh,
                    bias=bd_sbuf[:, :],
                )
                nc.sync.dma_start(out=outT[:, :], in_=outt[:, :])

    _cap_sync_waits(nc)
    return nc


def prepare_in_maps(inputs, Dynamic_L, W, Geo, KL, Wd, bd):
    """Host-side sharding + layout transforms (not counted in HW time)."""
    import ml_dtypes

    dt_in = ml_dtypes.bfloat16 if USE_BF16 else np.float32
    inputs = np.ascontiguousarray(inputs, dtype=np.float32)
    Dynamic_L = np.asarray(Dynamic_L, dtype=np.float32)
    W = np.asarray(W, dtype=np.float32)
    geot = np.ascontiguousarray(np.asarray(Geo, dtype=np.float32).T).astype(dt_in)
    klt = np.ascontiguousarray(np.asarray(KL, dtype=np.float32).T).astype(dt_in)
    wd = np.ascontiguousarray(np.asarray(Wd, dtype=np.float32))
    bdt = np.ascontiguousarray(np.asarray(bd, dtype=np.float32).reshape(UNITS, 1))

    in_maps = []
    for b in range(B):
        xaug = np.concatenate(
            [inputs[b], np.ones((N, 1), dtype=np.float32)], axis=1
        )  # [N, FA]
        xperm = np.ascontiguousarray(
            xaug.reshape(MT, P, FA).transpose(1, 0, 2).reshape(P, MT * FA)
        )
        in_maps.append(
            {
                "dlt": np.ascontiguousarray(Dynamic_L[b].T).astype(dt_in),
                "w0t": np.ascontiguousarray(W[b, :, :, 0].T).astype(dt_in),
                "w1t": np.ascontiguousarray(W[b, :, :, 1].T).astype(dt_in),
                "w2t": np.ascontiguousarray(W[b, :, :, 2].T).astype(dt_in),
                "geot": geot,
                "klt": klt,
                "xperm": xperm,
                "wd": wd,
                "bdt": bdt,
            }
        )
    return in_maps


_NC_CACHE = {}


def _get_nc(passes=1):
    if passes not in _NC_CACHE:
        _NC_CACHE[passes] = build_nc(passes=passes)
    return _NC_CACHE[passes]


def kernel(**inputs) -> np.ndarray:
    in_maps = prepare_in_maps(**inputs)
    nc = _get_nc(passes=1)
    res = run_bass_kernel_spmd(nc, in_maps, core_ids=list(range(B)))
    out = np.stack([res.results[b]["outT"].T for b in range(B)], axis=0)
    return np.ascontiguousarray(out, dtype=np.float32)


if __name__ == "__main__":
    rng = np.random.default_rng(0)
    ins = {
        "inputs": rng.standard_normal((B, N, F), dtype=np.float32),
        "Dynamic_L": rng.standard_normal((B, N, N), dtype=np.float32),
        "W": rng.random((B, N, N, 3), dtype=np.float32),
        "Geo": rng.standard_normal((N, N), dtype=np.float32),
        "KL": rng.standard_normal((N, N), dtype=np.float32),
        "Wd": rng.standard_normal((F, UNITS), dtype=np.float32) / 8.0,
        "bd": np.zeros(UNITS, dtype=np.float32),
    }
    out = kernel(**ins)
    print("out", out.shape, out.dtype)



# revision 16
# speedup vs baseline: 1.8252x; 1.0310x over previous
"""Trainium2 Bass kernel for the AHGCSP GCN layer problem.

Computes, per batch element b (8 total, one per NeuronCore):
    F   = Dynamic_L[b] * W[b,:,:,0] + Geo * W[b,:,:,1] + KL * W[b,:,:,2]
    P   = softmax(F, axis=-1)
    G1  = P @ inputs[b]
    out = tanh(G1 @ Wd + bd)

Formulation on device (everything transposed host-side, free for HW time):
  - Stream m-tiles of F^T = DL^T*W0^T + Geo^T*W1^T + KL^T*W2^T  [128 m, 2048 r]
  - P^T = exp(F^T)  (no max subtraction; F is bounded ~|8|)
  - G1T_aug[f',r] = sum_m Xaug[m,f'] * P^T[m,r] accumulated in PSUM, where
    Xaug = [inputs[b] | ones] so row 64 of G1T_aug is the softmax denominator.
  - 1/denom = exp(-ln(denom)) on ScalarE; broadcast across partitions via a
    K=1 matmul against a ones column; normalize on VectorE; second matmul with
    Wd stationary producing out^T [64 u, 2048 r]; tanh(+bd bias) on ScalarE.
  - host transposes out^T back.
"""

import numpy as np

import bass_rust
import concourse.bass as bass
import concourse.mybir as mybir
from concourse.tile import TileContext
from concourse.bass_utils import run_bass_kernel_spmd

B, N, F, UNITS = 8, 2048, 64, 64
P = 128            # partitions
MT = N // P        # m-tiles per core
FA = F + 1         # augmented feature dim (ones column)
NQ = 4             # moving-dim quarters (N / 512)
QW = N // NQ       # 512

FP32 = mybir.dt.float32
BF16 = mybir.dt.bfloat16
USE_BF16 = True          # cast the six big inputs (and X) to bf16 host-side
DT_IN = BF16 if USE_BF16 else FP32


def _cap_sync_waits(nc, max_waits=1):
    """The walrus build in this toolchain rejects instructions carrying more
    than a couple of sync waits ("Too many sync wait commands"). Hoist excess
    waits onto freshly inserted same-engine drain instructions immediately
    preceding the offender — identical blocking semantics, legal encoding."""
    eng_map = {
        mybir.EngineType.PE: nc.tensor,
        mybir.EngineType.DVE: nc.vector,
        mybir.EngineType.Activation: nc.scalar,
        mybir.EngineType.Pool: nc.gpsimd,
        mybir.EngineType.SP: nc.sync,
    }

    def _steal_fresh_drain(eng):
        binst = eng.drain()
        dmi = binst.ins
        for bb2 in nc.main_func.blocks:
            l2 = bb2.instructions
            if l2 and l2[-1].name == dmi.name:
                l2.pop()
                return dmi
        raise RuntimeError("could not find freshly appended drain")

    for bb in nc.main_func.blocks:
        il = bb.instructions
        i = 0
        while i < len(il):
            inst = il[i]
            si = inst.sync_info
            if si is not None and len(si.on_wait) > max_waits:
                waits = list(si.on_wait)
                extra, keep = waits[:-max_waits], waits[-max_waits:]
                eng = eng_map[inst.engine]
                for j in range(0, len(extra), max_waits):
                    dmi = _steal_fresh_drain(eng)
                    dmi.sync_info = bass_rust.SyncInfo(
                        on_wait=extra[j : j + max_waits], on_update=[]
                    )
                    il.insert(i, dmi)
                    i += 1
                inst.sync_info = bass_rust.SyncInfo(
                    on_wait=keep, on_update=list(si.on_update)
                )
            i += 1


def build_nc(passes: int = 1, in_bufs: int = 5, work_bufs: int = 2):
    """Build the per-core Bass graph. `passes` repeats the whole computation
    (for slope-based wall-clock timing); output is identical each pass."""
    nc = bass.Bass(num_devices=B)

    dlt = nc.declare_dram_parameter("dlt", [N, N], DT_IN, isOutput=False)
    w0t = nc.declare_dram_parameter("w0t", [N, N], DT_IN, isOutput=False)
    w1t = nc.declare_dram_parameter("w1t", [N, N], DT_IN, isOutput=False)
    w2t = nc.declare_dram_parameter("w2t", [N, N], DT_IN, isOutput=False)
    geot = nc.declare_dram_parameter("geot", [N, N], DT_IN, isOutput=False)
    klt = nc.declare_dram_parameter("klt", [N, N], DT_IN, isOutput=False)
    xperm = nc.declare_dram_parameter("xperm", [P, MT * FA], FP32, isOutput=False)
    wd = nc.declare_dram_parameter("wd", [F, UNITS], FP32, isOutput=False)
    bdt = nc.declare_dram_parameter("bdt", [UNITS, 1], FP32, isOutput=False)
    outT = nc.declare_dram_parameter("outT", [UNITS, N], FP32, isOutput=True)

    with TileContext(nc) as tc:
        with (
            tc.tile_pool(name="consts", bufs=1) as cpool,
            tc.tile_pool(name="ins", bufs=in_bufs) as ipool,
            tc.tile_pool(name="work", bufs=work_bufs) as wpool,
            tc.tile_pool(name="epi", bufs=1) as epool,
            tc.tile_pool(name="psum", bufs=1, space="PSUM") as ppool,
        ):
            x_sbuf = cpool.tile([P, MT * FA], FP32, tag="x")
            nc.sync.dma_start(out=x_sbuf[:, :], in_=xperm[:, :])
            wd_sbuf = cpool.tile([F, UNITS], FP32, tag="wd")
            nc.sync.dma_start(out=wd_sbuf[:, :], in_=wd[:, :])
            bd_sbuf = cpool.tile([UNITS, 1], FP32, tag="bd")
            nc.sync.dma_start(out=bd_sbuf[:, :], in_=bdt[:, :])
            ones_sb = cpool.tile([1, UNITS], FP32, tag="ones")
            nc.vector.memset(ones_sb[:, :], 1.0)

            for _ in range(passes):
                psum_g1 = ppool.tile([FA, N], FP32, tag="g1")
                for mi in range(MT):
                    rs = slice(P * mi, P * (mi + 1))
                    # pack DL|Geo|KL and W0|W1|W2 side by side so one wide
                    # tensor_tensor computes all three products
                    a3 = ipool.tile([P, 3 * N], DT_IN, tag="a3")
                    nc.sync.dma_start(out=a3[:, 0:N], in_=dlt[rs, :])
                    nc.sync.dma_start(out=a3[:, N : 2 * N], in_=geot[rs, :])
                    nc.sync.dma_start(out=a3[:, 2 * N : 3 * N], in_=klt[rs, :])
                    w3 = ipool.tile([P, 3 * N], DT_IN, tag="w3")
                    nc.sync.dma_start(out=w3[:, 0:N], in_=w0t[rs, :])
                    nc.sync.dma_start(out=w3[:, N : 2 * N], in_=w1t[rs, :])
                    nc.sync.dma_start(out=w3[:, 2 * N : 3 * N], in_=w2t[rs, :])

                    prod = wpool.tile([P, 3 * N], DT_IN, tag="prod")
                    nc.vector.tensor_mul(prod[:, :], a3[:, :], w3[:, :])
                    # accumulate the three products in place (frees an SBUF tag)
                    nc.vector.tensor_add(
                        prod[:, 0:N], prod[:, 0:N], prod[:, N : 2 * N]
                    )
                    nc.vector.tensor_add(
                        prod[:, 0:N], prod[:, 0:N], prod[:, 2 * N : 3 * N]
                    )

                    pt = wpool.tile([P, N], FP32, tag="pt")
                    nc.scalar.activation(
                        pt[:, :], prod[:, 0:N], mybir.ActivationFunctionType.Exp
                    )

                    xa = x_sbuf[:, FA * mi : FA * (mi + 1)]
                    for q in range(NQ):
                        nc.tensor.matmul(
                            psum_g1[:, QW * q : QW * (q + 1)],
                            xa,
                            pt[:, QW * q : QW * (q + 1)],
                            start=(mi == 0),
                            stop=(mi == MT - 1),
                        )

                # epilogue, pipelined in two r-halves so ACT/DVE/PE overlap:
                # recip = exp(-ln(denom)) on ScalarE straight from PSUM,
                # partition-broadcast via K=1 matmul, normalize, dense, tanh.
                H = N // 2
                for hh in range(2):
                    cs = slice(H * hh, H * (hh + 1))
                    g1t = epool.tile([F, H], FP32, tag="g1t")
                    nc.vector.tensor_copy(g1t[:, :], psum_g1[:F, cs])
                    lnd = epool.tile([1, H], FP32, tag="lnd")
                    nc.scalar.activation(
                        lnd[:, :],
                        psum_g1[F : F + 1, cs],
                        mybir.ActivationFunctionType.Ln,
                    )
                    recip = epool.tile([1, H], FP32, tag="recip")
                    nc.scalar.activation(
                        recip[:, :],
                        lnd[:, :],
                        mybir.ActivationFunctionType.Exp,
                        scale=-1.0,
                    )
                    psum_bc = ppool.tile([F, H], FP32, tag="bc")
                    for q in range(2):
                        nc.tensor.matmul(
                            psum_bc[:, QW * q : QW * (q + 1)],
                            ones_sb[:, :F],
                            recip[:, QW * q : QW * (q + 1)],
                            start=True,
                            stop=True,
                        )
                    g1n = epool.tile([F, H], FP32, tag="g1n")
                    nc.vector.tensor_mul(g1n[:, :], g1t[:, :], psum_bc[:, :])
                    psum_h = ppool.tile([UNITS, H], FP32, tag="h")
                    for q in range(2):
                        nc.tensor.matmul(
                            psum_h[:, QW * q : QW * (q + 1)],
                            wd_sbuf[:, :],
                            g1n[:, QW * q : QW * (q + 1)],
                            start=True,
                            stop=True,
                        )
                    outt = epool.tile([UNITS, H], FP32, tag="outt")
                    nc.scalar.activation(
                        outt[:, :],
                        psum_h[:, :],
                        mybir.ActivationFunctionType.Tanh,
                        bias=bd_sbuf[:, :],
                    )
                    nc.sync.dma_start(out=outT[:, cs], in_=outt[:, :])

    _cap_sync_waits(nc)
    return nc


def prepare_in_maps(inputs, Dynamic_L, W, Geo, KL, Wd, bd):
    """Host-side sharding + layout transforms (not counted in HW time)."""
    import ml_dtypes

    dt_in = ml_dtypes.bfloat16 if USE_BF16 else np.float32
    inputs = np.ascontiguousarray(inputs, dtype=np.float32)
    Dynamic_L = np.asarray(Dynamic_L, dtype=np.float32)
    W = np.asarray(W, dtype=np.float32)
    geot = np.ascontiguousarray(np.asarray(Geo, dtype=np.float32).T).astype(dt_in)
    klt = np.ascontiguousarray(np.asarray(KL, dtype=np.float32).T).astype(dt_in)
    wd = np.ascontiguousarray(np.asarray(Wd, dtype=np.float32))
    bdt = np.ascontiguousarray(np.asarray(bd, dtype=np.float32).reshape(UNITS, 1))

    in_maps = []
    for b in range(B):
        xaug = np.concatenate(
            [inputs[b], np.ones((N, 1), dtype=np.float32)], axis=1
        )  # [N, FA]
        xperm = np.ascontiguousarray(
            xaug.reshape(MT, P, FA).transpose(1, 0, 2).reshape(P, MT * FA)
        )
        in_maps.append(
            {
                "dlt": np.ascontiguousarray(Dynamic_L[b].T).astype(dt_in),
                "w0t": np.ascontiguousarray(W[b, :, :, 0].T).astype(dt_in),
                "w1t": np.ascontiguousarray(W[b, :, :, 1].T).astype(dt_in),
                "w2t": np.ascontiguousarray(W[b, :, :, 2].T).astype(dt_in),
                "geot": geot,
                "klt": klt,
                "xperm": xperm,
                "wd": wd,
                "bdt": bdt,
            }
        )
    return in_maps


_NC_CACHE = {}


def _get_nc(passes=1):
    if passes not in _NC_CACHE:
        _NC_CACHE[passes] = build_nc(passes=passes)
    return _NC_CACHE[passes]


def kernel(**inputs) -> np.ndarray:
    in_maps = prepare_in_maps(**inputs)
    nc = _get_nc(passes=1)
    res = run_bass_kernel_spmd(nc, in_maps, core_ids=list(range(B)))
    out = np.stack([res.results[b]["outT"].T for b in range(B)], axis=0)
    return np.ascontiguousarray(out, dtype=np.float32)


if __name__ == "__main__":
    rng = np.random.default_rng(0)
    ins = {
        "inputs": rng.standard_normal((B, N, F), dtype=np.float32),
        "Dynamic_L": rng.standard_normal((B, N, N), dtype=np.float32),
        "W": rng.random((B, N, N, 3), dtype=np.float32),
        "Geo": rng.standard_normal((N, N), dtype=np.float32),
        "KL": rng.standard_normal((N, N), dtype=np.float32),
        "Wd": rng.standard_normal((F, UNITS), dtype=np.float32) / 8.0,
        "bd": np.zeros(UNITS, dtype=np.float32),
    }
    out = kernel(**ins)
    print("out", out.shape, out.dtype)

